# revision 2
# baseline (speedup 1.0000x reference)
"""Trainium2 Bass kernel for ArcDecoder pair scoring.

Reference computation (N=768 nodes, H=128 features):
    pairs (i, j), i != j:  out[i,j] = W2 @ relu(W1a @ z_i + W1b @ z_j + b1) + b2
where W1 = [W1a | W1b] ([128, 256] split along input dim).

Strategy (pure data parallel over 8 NeuronCores, no collectives):
  - core c owns i-rows [96c, 96c+96); output rows are contiguous in the
    final pair-major ordering, so gather = concat + drop diagonal.
  - device layout puts the hidden dim k on SBUF partitions:
      Abias[k, a] = W1a @ z_loc.T + b1     (per-core [128, 96])
      Bt[k, j]    = W1b @ z.T              (replicated [128, 768], bf16)
    per a: H_a = relu(Bt + Abias[:, a])    (one DVE tensor_scalar, 4x mode)
    out[a, :] = W2 @ H_a + b2              (TensorE, zero-padded W2 stationary
                                            so row a lands on PSUM partition
                                            a%24; 24-row batches)
"""

import numpy as np
import ml_dtypes

import concourse.bass as bass
import concourse.tile as tile
from concourse import bacc, mybir
from concourse.bass_utils import run_bass_kernel_spmd

N = 768
H = 128
NCORES = 8
ROWS = N // NCORES          # 96 i-rows per core
BATCH = 24                  # output rows per PSUM batch
NBATCH = ROWS // BATCH      # 4
HALF = N // 2               # 384, PSUM bank limit for f32 is 512

_F32 = mybir.dt.float32
_BF16 = mybir.dt.bfloat16

_cache = {}


def _build():
    nc = bacc.Bacc(
        "TRN2", target_bir_lowering=False, debug=False, num_devices=NCORES
    )

    zT_d = nc.dram_tensor("zT", [H, N], _F32, kind="ExternalInput")
    zTloc_d = nc.dram_tensor("zTloc", [H, ROWS], _F32, kind="ExternalInput")
    w1aT_d = nc.dram_tensor("w1aT", [H, H], _F32, kind="ExternalInput")
    w1bT_d = nc.dram_tensor("w1bT", [H, H], _F32, kind="ExternalInput")
    b1col_d = nc.dram_tensor("b1col", [H, 1], _F32, kind="ExternalInput")
    b2col_d = nc.dram_tensor("b2col", [BATCH, 1], _F32, kind="ExternalInput")
    S_d = nc.dram_tensor("S", [H, ROWS, BATCH], _BF16, kind="ExternalInput")
    out_d = nc.dram_tensor("out", [ROWS, N], _F32, kind="ExternalOutput")

    relu = mybir.ActivationFunctionType.Relu  # noqa: F841
    ident = mybir.ActivationFunctionType.Identity
    copyf = mybir.ActivationFunctionType.Copy
    add_op = mybir.AluOpType.add
    max_op = mybir.AluOpType.max

    with tile.TileContext(nc) as tc:
        with (
            tc.tile_pool(name="const", bufs=1) as cpool,
            tc.tile_pool(name="hpool", bufs=4) as hpool,
            tc.tile_pool(name="opool", bufs=2) as opool,
            tc.tile_pool(name="psA", bufs=1, space=bass.MemorySpace.PSUM) as psA,
            tc.tile_pool(name="psum", bufs=2, space=bass.MemorySpace.PSUM) as pspool,
        ):
            zT_sb = cpool.tile([H, N], _F32)
            nc.sync.dma_start(zT_sb[:], zT_d[:])
            zTloc_sb = cpool.tile([H, ROWS], _F32)
            nc.sync.dma_start(zTloc_sb[:], zTloc_d[:])
            w1aT_sb = cpool.tile([H, H], _F32)
            nc.sync.dma_start(w1aT_sb[:], w1aT_d[:])
            w1bT_sb = cpool.tile([H, H], _F32)
            nc.sync.dma_start(w1bT_sb[:], w1bT_d[:])
            b1col_sb = cpool.tile([H, 1], _F32)
            nc.sync.dma_start(b1col_sb[:], b1col_d[:])
            b2col_sb = cpool.tile([BATCH, 1], _F32)
            nc.sync.dma_start(b2col_sb[:], b2col_d[:])
            S_sb = []
            for b in range(NBATCH):
                t = cpool.tile([H, BATCH, BATCH], _BF16, tag=f"S{b}")
                nc.sync.dma_start(
                    t[:], S_d[:, b * BATCH : (b + 1) * BATCH, :]
                )
                S_sb.append(t)

            # Abias[k, a] = W1a @ z_loc.T + b1
            at_ps = psA.tile([H, ROWS], _F32, tag="at")
            nc.tensor.matmul(at_ps[:], w1aT_sb[:], zTloc_sb[:], start=True, stop=True)
            abias_sb = cpool.tile([H, ROWS], _F32)
            nc.vector.tensor_scalar_add(abias_sb[:], at_ps[:], b1col_sb[:])

            # Bt[k, j] = W1b @ z.T  (f32 matmul, converted to bf16 in SBUF)
            bt_sb = cpool.tile([H, N], _BF16)
            for h in range(2):
                bt_ps = psA.tile([H, HALF], _F32, tag=f"bt{h}")
                nc.tensor.matmul(
                    bt_ps[:],
                    w1bT_sb[:],
                    zT_sb[:, h * HALF : (h + 1) * HALF],
                    start=True,
                    stop=True,
                )
                nc.scalar.activation(
                    bt_sb[:, h * HALF : (h + 1) * HALF], bt_ps[:], copyf
                )

            for b in range(NBATCH):
                ps0 = pspool.tile([BATCH, HALF], _F32, tag="ps0")
                ps1 = pspool.tile([BATCH, HALF], _F32, tag="ps1")
                for r in range(BATCH):
                    a = b * BATCH + r
                    ht = hpool.tile([H, N], _BF16, tag="H")
                    nc.vector.tensor_scalar(
                        ht[:], bt_sb[:], abias_sb[:, a : a + 1], 0.0, add_op, max_op
                    )
                    st = S_sb[b][:, r, :]
                    first = r == 0
                    last = r == BATCH - 1
                    nc.tensor.matmul(
                        ps0[:], st, ht[:, 0:HALF], start=first, stop=last
                    )
                    nc.tensor.matmul(
                        ps1[:], st, ht[:, HALF:N], start=first, stop=last
                    )
                ot = opool.tile([BATCH, N], _F32, tag="ot")
                nc.scalar.activation(
                    ot[:, 0:HALF], ps0[:], ident, bias=b2col_sb[:], scale=1.0
                )
                nc.scalar.activation(
                    ot[:, HALF:N], ps1[:], ident, bias=b2col_sb[:], scale=1.0
                )
                nc.sync.dma_start(out_d[b * BATCH : (b + 1) * BATCH, :], ot[:])

    nc.compile()
    return nc


def _get_nc():
    if "nc" not in _cache:
        _cache["nc"] = _build()
    return _cache["nc"]


def _prep_in_maps(z, W1, b1, W2, b2):
    z = np.asarray(z, np.float32)
    W1 = np.asarray(W1, np.float32)
    b1 = np.asarray(b1, np.float32)
    W2 = np.asarray(W2, np.float32)
    b2 = np.asarray(b2, np.float32)

    zT = np.ascontiguousarray(z.T)                     # [H, N]
    w1aT = np.ascontiguousarray(W1[:, :H].T)           # [c, k]
    w1bT = np.ascontiguousarray(W1[:, H:].T)           # [c, k]
    b1col = np.ascontiguousarray(b1.reshape(H, 1))
    b2col = np.full((BATCH, 1), float(b2[0]), np.float32)

    # zero-padded stationary: S[k, a, a % BATCH] = W2[0, k]
    S = np.zeros((H, ROWS, BATCH), np.float32)
    ar = np.arange(ROWS)
    S[:, ar, ar % BATCH] = W2[0][:, None]
    S = S.astype(ml_dtypes.bfloat16)

    in_maps = []
    for c in range(NCORES):
        in_maps.append(
            {
                "zT": zT,
                "zTloc": np.ascontiguousarray(zT[:, c * ROWS : (c + 1) * ROWS]),
                "w1aT": w1aT,
                "w1bT": w1bT,
                "b1col": b1col,
                "b2col": b2col,
                "S": S,
            }
        )
    return in_maps


def _assemble(results):
    full = np.concatenate(
        [np.asarray(results[c]["out"], np.float32) for c in range(NCORES)], axis=0
    )  # [N, N] scores incl. diagonal
    mask = ~np.eye(N, dtype=bool)
    return full[mask]  # pair-major order: i-major, j ascending, j != i


def run(z, W1, b1, W2, b2, trace=False, tmpdir=None):
    nc = _get_nc()
    in_maps = _prep_in_maps(z, W1, b1, W2, b2)
    res = run_bass_kernel_spmd(
        nc, in_maps, core_ids=list(range(NCORES)), trace=trace, tmpdir=tmpdir
    )
    return _assemble(res.results), res


def kernel(z, W1, b1, W2, b2):
    out, _ = run(z, W1, b1, W2, b2, trace=False)
    return out


# revision 5
# speedup vs baseline: 1.1669x; 1.1669x over previous
"""Trainium2 Bass kernel for ArcDecoder pair scoring.

Reference computation (N=768 nodes, H=128 features):
    pairs (i, j), i != j:  out[i,j] = W2 @ relu(W1a @ z_i + W1b @ z_j + b1) + b2
where W1 = [W1a | W1b] ([128, 256] split along input dim).

Strategy (pure data parallel over 8 NeuronCores, no collectives):
  - core c owns i-rows [96c, 96c+96); output rows are contiguous in the
    final pair-major ordering, so gather = concat + drop diagonal.
  - device layout puts the hidden dim k on SBUF partitions:
      Abias[k, a] = W1a @ z_loc.T + b1     (per-core [128, 96])
      Bt[k, j]    = W1b @ z.T              (replicated [128, 768], bf16)
    per a: H_a = relu(Bt + Abias[:, a])    (DVE tensor_scalar 2:1 with ACT
                                            activation(Relu, bias))
    out[a, :] = W2 @ H_a + b2              (TensorE: zero-padded W2
                                            stationary, 3 col-groups run
                                            concurrently; row for a = 3r+g
                                            lands on PSUM partition 32g+r)
"""

import numpy as np
import ml_dtypes

import concourse.bass as bass
import concourse.tile as tile
from concourse import bacc, mybir
from concourse.bass_utils import run_bass_kernel_spmd

N = 768
H = 128
NCORES = 8
ROWS = N // NCORES          # 96 i-rows per core
NGRP = 3                    # PE column groups (PSUM partitions 32g..32g+31)
RND = ROWS // NGRP          # 32 rounds; round r, group g handles a = 3r + g
HALF = N // 2               # 384, PSUM bank limit for f32 is 512

_F32 = mybir.dt.float32
_BF16 = mybir.dt.bfloat16

_cache = {}


def _build():
    nc = bacc.Bacc(
        "TRN2",
        target_bir_lowering=False,
        debug=False,
        enable_asserts=False,
        num_devices=NCORES,
    )

    zT_d = nc.dram_tensor("zT", [H, N], _BF16, kind="ExternalInput")
    zTloc_d = nc.dram_tensor("zTloc", [H, ROWS], _BF16, kind="ExternalInput")
    w1aT_d = nc.dram_tensor("w1aT", [H, H], _BF16, kind="ExternalInput")
    w1bT_d = nc.dram_tensor("w1bT", [H, H], _BF16, kind="ExternalInput")
    b1col_d = nc.dram_tensor("b1col", [H, 1], _F32, kind="ExternalInput")
    b2col_d = nc.dram_tensor("b2col", [ROWS, 1], _F32, kind="ExternalInput")
    S_d = nc.dram_tensor("S", [H, RND, 32], _BF16, kind="ExternalInput")
    out_d = nc.dram_tensor("out", [ROWS, N], _F32, kind="ExternalOutput")

    relu = mybir.ActivationFunctionType.Relu
    add_op = mybir.AluOpType.add
    max_op = mybir.AluOpType.max

    with tile.TileContext(nc) as tc:
        with (
            tc.tile_pool(name="const", bufs=1) as cpool,
            tc.tile_pool(name="hpool", bufs=9) as hpool,
            tc.tile_pool(name="opool", bufs=1) as opool,
            tc.tile_pool(name="psA", bufs=1, space=bass.MemorySpace.PSUM) as psA,
            tc.tile_pool(name="psum", bufs=1, space=bass.MemorySpace.PSUM) as pspool,
        ):
            # ACT spline-table prewarm: a dummy relu so the one-time
            # ACT_TABLE_LOAD overlaps the input DMAs.
            scratch = cpool.tile([1, 8], _F32, tag="scratch")
            nc.gpsimd.memset(scratch[:], 0.0)
            nc.scalar.activation(scratch[:], scratch[:], relu)

            # inputs, spread across engine DGE queues
            zT_sb = cpool.tile([H, N], _BF16)
            nc.sync.dma_start(zT_sb[:], zT_d[:])
            zTloc_sb = cpool.tile([H, ROWS], _BF16)
            nc.scalar.dma_start(zTloc_sb[:], zTloc_d[:])
            w1aT_sb = cpool.tile([H, H], _BF16)
            nc.scalar.dma_start(w1aT_sb[:], w1aT_d[:])
            w1bT_sb = cpool.tile([H, H], _BF16)
            nc.sync.dma_start(w1bT_sb[:], w1bT_d[:])
            b1col_sb = cpool.tile([H, 1], _F32)
            nc.scalar.dma_start(b1col_sb[:], b1col_d[:])
            b2col_sb = cpool.tile([ROWS, 1], _F32)
            nc.scalar.dma_start(b2col_sb[:], b2col_d[:])
            S_sb = cpool.tile([H, RND, 32], _BF16)
            nc.gpsimd.dma_start(S_sb[:], S_d[:])

            # Abias[k, a] = W1a @ z_loc.T + b1
            at_ps = psA.tile([H, ROWS], _F32, tag="at")
            nc.tensor.matmul(at_ps[:], w1aT_sb[:], zTloc_sb[:], start=True, stop=True)
            abias_sb = cpool.tile([H, ROWS], _F32)
            nc.vector.tensor_scalar_add(abias_sb[:], at_ps[:], b1col_sb[:])

            # Bt[k, j] = W1b @ z.T   (bf16 in SBUF)
            bt_sb = cpool.tile([H, N], _BF16)
            for h in range(2):
                bt_ps = psA.tile([H, HALF], _F32, tag=f"bt{h}")
                nc.tensor.matmul(
                    bt_ps[:],
                    w1bT_sb[:],
                    zT_sb[:, h * HALF : (h + 1) * HALF],
                    start=True,
                    stop=True,
                )
                nc.vector.tensor_copy(
                    bt_sb[:, h * HALF : (h + 1) * HALF], bt_ps[:]
                )

            ps = [
                pspool.tile([ROWS, HALF], _F32, tag=f"ps{h}", name=f"ps{h}")
                for h in range(2)
            ]
            for r in range(RND):
                hts = []
                for g in range(NGRP):
                    a = NGRP * r + g
                    ht = hpool.tile([H, N], _BF16, tag="H")
                    if g < 2:
                        nc.vector.tensor_scalar(
                            ht[:], bt_sb[:], abias_sb[:, a : a + 1], 0.0,
                            add_op, max_op,
                        )
                    else:
                        nc.scalar.activation(
                            ht[:], bt_sb[:], relu,
                            bias=abias_sb[:, a : a + 1], scale=1.0,
                        )
                    hts.append(ht)
                first = r == 0
                last = r == RND - 1
                for h in range(2):
                    for g in range(NGRP):
                        nc.tensor.matmul(
                            ps[h][32 * g : 32 * g + 32, :],
                            S_sb[:, r, :],
                            hts[g][:, h * HALF : (h + 1) * HALF],
                            start=first,
                            stop=last,
                        )

            # evict: out rows p = 32g + r  ->  DRAM row a = 3r + g
            ot = opool.tile([ROWS, N], _F32, tag="ot")
            for h in range(2):
                nc.vector.tensor_scalar_add(
                    ot[:, h * HALF : (h + 1) * HALF], ps[h][:], b2col_sb[:]
                )
            out_view = out_d.ap().rearrange("(r three) n -> three r n", three=NGRP)
            for g in range(NGRP):
                nc.sync.dma_start(out_view[g], ot[32 * g : 32 * g + 32, :])

    nc.compile()
    return nc


def _get_nc():
    if "nc" not in _cache:
        _cache["nc"] = _build()
    return _cache["nc"]


def _prep_in_maps(z, W1, b1, W2, b2):
    z = np.asarray(z, np.float32)
    W1 = np.asarray(W1, np.float32)
    b1 = np.asarray(b1, np.float32)
    W2 = np.asarray(W2, np.float32)
    b2 = np.asarray(b2, np.float32)

    bf = ml_dtypes.bfloat16
    zT = np.ascontiguousarray(z.T)                          # [H, N]
    w1aT = np.ascontiguousarray(W1[:, :H].T).astype(bf)     # [c, k]
    w1bT = np.ascontiguousarray(W1[:, H:].T).astype(bf)     # [c, k]
    b1col = np.ascontiguousarray(b1.reshape(H, 1))
    # output row for a sits at PSUM partition 32*(a%3) + a//3; the bias is
    # uniform so a plain [ROWS, 1] fill works for any permutation.
    b2col = np.full((ROWS, 1), float(b2[0]), np.float32)

    # zero-padded stationary (shared by the 3 col groups):
    # S[k, r, r] = W2[0, k]
    S = np.zeros((H, RND, 32), np.float32)
    r = np.arange(RND)
    S[:, r, r] = W2[0][:, None]
    S = S.astype(bf)

    zT_bf = zT.astype(bf)
    in_maps = []
    for c in range(NCORES):
        in_maps.append(
            {
                "zT": zT_bf,
                "zTloc": np.ascontiguousarray(
                    zT[:, c * ROWS : (c + 1) * ROWS]
                ).astype(bf),
                "w1aT": w1aT,
                "w1bT": w1bT,
                "b1col": b1col,
                "b2col": b2col,
                "S": S,
            }
        )
    return in_maps


def _assemble(results):
    # device row p of core c maps to global i = c*ROWS + 3*(p%32) + p//32
    perm = np.argsort(
        np.arange(ROWS) % 32 * NGRP + np.arange(ROWS) // 32, kind="stable"
    )  # not used; rows are already unpermuted by the strided output DMA
    del perm
    full = np.concatenate(
        [np.asarray(results[c]["out"], np.float32) for c in range(NCORES)], axis=0
    )  # [N, N] scores incl. diagonal
    mask = ~np.eye(N, dtype=bool)
    return full[mask]  # pair-major order: i-major, j ascending, j != i


def run(z, W1, b1, W2, b2, trace=False, tmpdir=None):
    nc = _get_nc()
    in_maps = _prep_in_maps(z, W1, b1, W2, b2)
    res = run_bass_kernel_spmd(
        nc, in_maps, core_ids=list(range(NCORES)), trace=trace, tmpdir=tmpdir
    )
    return _assemble(res.results), res


def kernel(z, W1, b1, W2, b2):
    out, _ = run(z, W1, b1, W2, b2, trace=False)
    return out
